# revision 51
# baseline (speedup 1.0000x reference)
"""Distributed multi-head attention kernel for one TRN2 chip (8 NeuronCores).

Problem: B=4, N=2048, C=1024, H=16 heads (hd=64), fp32 in/out.
  qkv = x @ W_qkv.T ; per-head scores = q k^T * hd^-0.5 + global_bias
  attn = softmax(scores) ; out = attn @ v ; y = out @ W_proj.T + b_proj

Sharding: head-parallel — core c owns heads {2c, 2c+1} for all batches and
computes qkv projection (its W_qkv rows), attention, and the unnormalized
attention output for its heads over all 8192 tokens.  A single bf16 AllToAll
then redistributes from head-parallel to token-parallel ([8 token slices] x
[128 channels] blocks), after which each core computes the final projection
for its 1024-token slice against the full W_proj.

Everything stays transposed (channels on SBUF partitions) end to end:
  xt [C, B*N], qT/kT [128(2 heads*64), N], v [N, 64] (+ ones column for the
  softmax denominator), out^T [128, B*N], final^T [C, 1024-token slice].
The host prepares transposed/bf16 inputs and untransposes the output;
softmax uses exp(s*scale + b) = exp(s*scale) * eb with eb = exp(bias)
precomputed on the host, so no bias-add pass is needed on-chip.
"""

import numpy as np
import ml_dtypes

import concourse.mybir as mybir
import concourse.tile as tile
from concourse import bacc
from concourse.bass_utils import run_bass_kernel_spmd


def _patch_act_tables():
    """This kernel uses Exp and Ln; by default the table-load pass resolves
    Exp to the `exp_and_others` set and Ln to `natural_log_exp_and_others`,
    thrashing table loads (~1.3us each) between the two.  Hide Exp/the other
    shared fns from every set except `natural_log_exp_and_others` (which has
    both) so a single table load serves the whole kernel."""
    import concourse.hw_specs as hw_specs

    if getattr(bacc, "_act_tables_patched", False):
        return
    orig = hw_specs.get_activation_tables

    def patched(module_arch):
        tables = orig(module_arch)
        keep = tables.get("natural_log_exp_and_others")
        if keep:
            e = mybir.ActivationFunctionType.Exp
            for name, fns in tables.items():
                if name != "natural_log_exp_and_others":
                    fns.discard(e)
        return tables

    bacc.get_activation_tables = patched
    bacc._act_tables_patched = True


_patch_act_tables()

F32 = mybir.dt.float32
BF16 = mybir.dt.bfloat16
BF16_NP = ml_dtypes.bfloat16

N_CORES = 8
B, N, C = 4, 2048, 1024
H = 16
HD = C // H          # 64
SCALE = HD ** -0.5
TOK = B * N          # 8192
TSLICE = TOK // N_CORES  # 1024 tokens per core for the final projection
NCT = C // 128       # 8 c-tiles
NKT = N // 128       # 16 k-tiles per batch
NQC = N // 512       # 4 q-chunks per batch
GK = 2               # k-tiles per exp group
TB = TSLICE // B     # 256 tokens per (core, batch) in the final output

_GRAPH = None


def _build():
    nc = bacc.Bacc("TRN2", target_bir_lowering=False, debug=False,
                   num_devices=N_CORES)

    xt = nc.declare_dram_parameter("xt", [C, TOK], BF16, isOutput=False)
    wq = nc.declare_dram_parameter("wq", [C, 128], BF16, isOutput=False)
    wk = nc.declare_dram_parameter("wk", [C, 128], BF16, isOutput=False)
    wv = nc.declare_dram_parameter("wv", [C, 130], BF16, isOutput=False)
    wp = nc.declare_dram_parameter("wp", [C, C], BF16, isOutput=False)
    bp = nc.declare_dram_parameter("bp", [C, 1], F32, isOutput=False)
    eb = nc.declare_dram_parameter("eb", [128, NKT, N], BF16, isOutput=False)
    out = nc.declare_dram_parameter("out", [C, TSLICE], F32, isOutput=True)

    xt_r = xt.rearrange("(ct p) t -> p ct t", p=128)
    wq_r = wq.rearrange("(ct p) f -> p ct f", p=128)
    wk_r = wk.rearrange("(ct p) f -> p ct f", p=128)
    wv_r = wv.rearrange("(ct p) f -> p ct f", p=128)
    wp_r = wp.rearrange("(ct p) o -> p ct o", p=128)
    bp_r = bp.rearrange("(ot p) one -> p ot one", p=128)

    with tile.TileContext(nc) as tc:
        with (
            tc.tile_pool(name="const", bufs=1) as cpool,
            tc.tile_pool(name="xt", bufs=1) as xpool,
            tc.tile_pool(name="qk", bufs=2) as qkpool,
            tc.tile_pool(name="vv", bufs=2) as vpool,
            tc.tile_pool(name="pp", bufs=5) as ppool,
            tc.tile_pool(name="outu", bufs=1) as opool,
            tc.tile_pool(name="den", bufs=1) as dpool,
            tc.tile_pool(name="nrm", bufs=1) as npool,
            tc.tile_pool(name="outn", bufs=1) as onpool,
            tc.tile_pool(name="fin", bufs=2) as fpool,
            tc.tile_pool(name="gat", bufs=1) as gpool,
            tc.tile_pool(name="dram", bufs=1, space="DRAM") as drpool,
            tc.tile_pool(name="ps_s", bufs=2, space="PSUM") as ps_s,
            tc.tile_pool(name="ps_o", bufs=2, space="PSUM") as ps_o,
            tc.tile_pool(name="ps_m", bufs=2, space="PSUM") as ps_m,
        ):
            # ---- resident constants -------------------------------------
            # qkv weights first: batch 0's qkv is the critical path at start;
            # eb/wp are not needed until attention / the first projection
            wq_t = cpool.tile([128, NCT, 128], BF16, tag="wq")
            wk_t = cpool.tile([128, NCT, 128], BF16, tag="wk")
            wv_t = cpool.tile([128, NCT, 130], BF16, tag="wv")
            nc.sync.dma_start(wq_t[:], wq_r)
            nc.sync.dma_start(wk_t[:], wk_r)
            nc.sync.dma_start(wv_t[:], wv_r)
            bp_t = cpool.tile([128, NCT, 1], F32, tag="bp")
            nc.sync.dma_start(bp_t[:], bp_r)
            ones_t = cpool.tile([1, 64], F32, tag="ones")
            nc.gpsimd.memset(ones_t[:], 1.0)
            eb_ts = []
            for j in range(NKT):
                ebj = cpool.tile([128, N], BF16, tag=f"eb{j}")
                eb_ts.append(ebj)
            wp_t = cpool.tile([128, NCT, C], BF16, tag="wp")

            # warmup collective: absorb the first-call ENCD/NCCL staging
            # latency (~40us) while the initial DMAs and QKV run
            wu_i = drpool.tile([N_CORES, 128, TB], BF16, tag="wu_i")
            wu_o = drpool.tile([N_CORES, 128, TB], BF16, tag="wu_o")
            wz = cpool.tile([128, TB], BF16, tag="wz")
            nc.gpsimd.memset(wz[:], 0.0)
            nc.sync.dma_start(wu_i[0, :, :], wz[:])
            for _wu in range(2):
                nc.gpsimd.collective_compute(
                    "AllToAll",
                    mybir.AluOpType.bypass,
                    replica_groups=[list(range(N_CORES))],
                    ins=[wu_i.opt()],
                    outs=[wu_o.opt()],
                )

            def emit_proj(pb_, a2a_o_):
                """Gather this batch's AllToAll result and project it.
                Emitted one batch late so the collective latency hides under
                the next batch's attention."""
                gat = gpool.tile([128, NCT, TB], BF16, tag="gat")
                for ct in range(NCT):
                    nc.sync.dma_start(gat[:, ct, :], a2a_o_[ct, :, :])
                for ot in range(NCT):
                    pf = ps_m.tile([128, TB], F32, tag="ps_m")
                    for ct in range(NCT):
                        nc.tensor.matmul(
                            pf[:],
                            wp_t[:, ct, ot * 128:(ot + 1) * 128],
                            gat[:, ct, :],
                            start=(ct == 0), stop=(ct == NCT - 1),
                        )
                    fin = fpool.tile([128, TB], F32, tag="fin")
                    nc.vector.tensor_scalar_add(fin[:], pf[:], bp_t[:, ot, :])
                    nc.sync.dma_start(
                        out[ot * 128:(ot + 1) * 128,
                            pb_ * TB:(pb_ + 1) * TB],
                        fin[:],
                    )

            pending_proj = None

            xt_tiles = {}

            def load_xt(bb):
                xt_t = xpool.tile([128, NCT, N], BF16, tag="xt")
                for ct in range(NCT):
                    nc.sync.dma_start(
                        xt_t[:, ct, :], xt_r[:, ct, bb * N:(bb + 1) * N]
                    )
                xt_tiles[bb] = xt_t

            load_xt(0)
            # eb is not needed until attention; issue its DMAs behind
            # batch 0's critical loads, one independent tile per k-tile so
            # each P-multiply waits only on its own chunk
            for j in range(NKT):
                nc.gpsimd.dma_start(eb_ts[j][:], eb[:, j, :])

            qkv_tiles = {}

            def alloc_qkv(bb):
                qT = qkpool.tile([128, N], BF16, tag="qT")
                kT = qkpool.tile([128, N], BF16, tag="kT")
                v_t = vpool.tile([128, NKT, 130], BF16, tag="vv")
                qkv_tiles[bb] = (qT, kT, v_t)

            def qkv_chunks(bb):
                """Yield thunks, each emitting one 8-matmul qkv chain for
                batch bb.  Emitted interleaved into the previous batch's
                attention so the PE always has dense independent work."""
                qT, kT, v_t = qkv_tiles[bb]
                xt_t = xt_tiles[bb]

                def qk_chain(dst, w_t, tcn):
                    pqk = ps_m.tile([128, 512], F32, tag="ps_m")
                    for ct in range(NCT):
                        nc.tensor.matmul(
                            pqk[:],
                            w_t[:, ct, :],
                            xt_t[:, ct, tcn * 512:(tcn + 1) * 512],
                            start=(ct == 0), stop=(ct == NCT - 1),
                        )
                    nc.vector.tensor_copy(
                        dst[:, tcn * 512:(tcn + 1) * 512], pqk[:]
                    )

                def v_chain(kt):
                    # v (+ones cols): head slices [0:65]=[v_h0|ones] and
                    # [65:130]=[v_h1|ones] put both denominators at psum
                    # row 64
                    pv = ps_m.tile([128, 512], F32, tag="ps_m")
                    for ct in range(NCT):
                        nc.tensor.matmul(
                            pv[:, 0:130],
                            xt_t[:, ct, kt * 128:(kt + 1) * 128],
                            wv_t[:, ct, :],
                            start=(ct == 0), stop=(ct == NCT - 1),
                        )
                    nc.vector.tensor_copy(v_t[:, kt, :], pv[:, 0:130])
                    nc.gpsimd.memset(v_t[:, kt, 64:65], 1.0)
                    nc.gpsimd.memset(v_t[:, kt, 129:130], 1.0)

                for dst, w_t in ((qT, wq_t), (kT, wk_t)):
                    for tcn in range(NQC):
                        yield lambda d=dst, w=w_t, t=tcn: qk_chain(d, w, t)
                for kt in range(NKT):
                    yield lambda k=kt: v_chain(k)

            # batch 0's qkv runs up front (nothing to overlap with)
            alloc_qkv(0)
            for chunk in qkv_chunks(0):
                chunk()
            if B > 1:
                load_xt(1)
                alloc_qkv(1)
            # wp is needed only by the first projection (mid batch 1)
            for ct in range(NCT):
                nc.gpsimd.dma_start(wp_t[:, ct, :], wp_r[:, ct, :])

            # ---- per-batch attention ------------------------------------
            for b in range(B):
                qT, kT, v_t = qkv_tiles.pop(b)
                xt_tiles.pop(b)
                filler = iter(qkv_chunks(b + 1)) if b + 1 < B else iter(())

                def emit_filler(n=1):
                    for _ in range(n):
                        ch = next(filler, None)
                        if ch is not None:
                            ch()

                ln0 = dpool.tile([1, N], F32, tag="ln0")
                ln1 = dpool.tile([1, N], F32, tag="ln1")
                rc0 = dpool.tile([1, N], F32, tag="rc0")
                rc1 = dpool.tile([1, N], F32, tag="rc1")
                ou = opool.tile([128, N], BF16, tag="outu")
                bc = npool.tile([128, N], BF16, tag="bc")
                on = onpool.tile([128, N], BF16, tag="outn")
                a2a_i = drpool.tile([N_CORES, 128, TB], BF16, tag=f"a2ai{b}")
                a2a_o = drpool.tile([N_CORES, 128, TB], BF16, tag=f"a2ao{b}")
                # Both heads processed together per k-tile: the two K=64
                # score matmuls occupy disjoint PE row groups (partitions
                # 0-63 / 64-127) and run concurrently.  attn@v matmuls are
                # skewed one k-tile behind the scores so the PE never waits
                # on the exp->mul chain of the current k-tile.
                for qc in range(NQC):
                    q0 = qc * 512
                    po0 = ps_o.tile([65, 512], F32, tag="ps_o")
                    po1 = ps_o.tile([65, 512], F32, tag="ps_o")
                    po = [po0, po1]
                    pend = []
                    for kt in range(NKT):
                        ps = ps_s.tile([128, 1024], F32, tag="ps_s")
                        for h in range(2):
                            nc.tensor.matmul(
                                ps[:, h * 512:(h + 1) * 512],
                                kT[h * 64:h * 64 + 64,
                                   kt * 128:(kt + 1) * 128],
                                qT[h * 64:h * 64 + 64, q0:q0 + 512],
                                start=True, stop=True,
                            )
                        if len(pend) >= 2:
                            pkt, ppw = pend.pop(0)
                            for h in range(2):
                                nc.tensor.matmul(
                                    po[h][:],
                                    v_t[:, pkt, h * 65:h * 65 + 65],
                                    ppw[:, h * 512:(h + 1) * 512],
                                    start=(pkt == 0), stop=False,
                                )
                        pexp = ppool.tile([128, 1024], BF16, tag="pp")
                        nc.scalar.activation(
                            pexp[:], ps[:],
                            mybir.ActivationFunctionType.Exp, scale=SCALE,
                        )
                        pw = ppool.tile([128, 1024], BF16, tag="pp")
                        ebs = eb_ts[kt][:, q0:q0 + 512].unsqueeze(1)
                        ebs = ebs.broadcast_to([128, 2, 512])
                        nc.vector.tensor_mul(
                            pw[:].rearrange("p (a b) -> p a b", a=2),
                            pexp[:].rearrange("p (a b) -> p a b", a=2),
                            ebs,
                        )
                        pend.append((kt, pw))
                        # interleave next batch's qkv chains as PE filler
                        if (qc == 0 and kt >= 9 and kt % 2 == 1) or \
                           (qc > 0 and kt % 3 == 2):
                            emit_filler(1)
                    for pkt, ppw in pend:
                        for h in range(2):
                            nc.tensor.matmul(
                                po[h][:],
                                v_t[:, pkt, h * 65:h * 65 + 65],
                                ppw[:, h * 512:(h + 1) * 512],
                                start=False, stop=(pkt == NKT - 1),
                            )
                    for h, lnd, rcd in ((0, ln0, rc0), (1, ln1, rc1)):
                        nc.vector.tensor_copy(
                            ou[h * 64:(h + 1) * 64, q0:q0 + 512],
                            po[h][0:64, :],
                        )
                        nc.scalar.activation(
                            lnd[0:1, q0:q0 + 512], po[h][64:65, :],
                            mybir.ActivationFunctionType.Ln,
                        )
                        # 1/den = exp(-ln den); broadcast over the head's 64
                        # partitions now so nothing stalls at batch end
                        nc.scalar.activation(
                            rcd[0:1, q0:q0 + 512], lnd[0:1, q0:q0 + 512],
                            mybir.ActivationFunctionType.Exp, scale=-1.0,
                        )
                        pb = ps_o.tile([64, 512], F32, tag="ps_o")
                        nc.tensor.matmul(
                            pb[:], ones_t[:], rcd[0:1, q0:q0 + 512],
                            start=True, stop=True,
                        )
                        nc.vector.tensor_copy(
                            bc[h * 64:(h + 1) * 64, q0:q0 + 512], pb[:]
                        )
                    nc.vector.tensor_mul(
                        on[:, q0:q0 + 512], ou[:, q0:q0 + 512],
                        bc[:, q0:q0 + 512],
                    )
                    for j in (2 * qc, 2 * qc + 1):
                        nc.sync.dma_start(
                            a2a_i[j, :, :], on[:, j * TB:(j + 1) * TB]
                        )
                    if qc == 1 and pending_proj is not None:
                        # previous batch's projection: its AllToAll has had
                        # a full qkv stage + two q-chunks to complete
                        emit_proj(*pending_proj)
                        pending_proj = None
                    emit_filler(2)

                emit_filler(NKT + 2 * NQC)  # flush any remaining chunks
                nc.gpsimd.collective_compute(
                    "AllToAll",
                    mybir.AluOpType.bypass,
                    replica_groups=[list(range(N_CORES))],
                    ins=[a2a_i.opt()],
                    outs=[a2a_o.opt()],
                )
                pending_proj = (b, a2a_o)
                if b + 2 < B:
                    load_xt(b + 2)
                    alloc_qkv(b + 2)

            emit_proj(*pending_proj)
    nc.compile()
    return nc


def _graph():
    global _GRAPH
    if _GRAPH is None:
        _GRAPH = _build()
    return _GRAPH


def _prep_inputs(x, W_qkv, W_proj, b_proj, global_bias):
    x = np.asarray(x, dtype=np.float32)
    W_qkv = np.asarray(W_qkv, dtype=np.float32)
    W_proj = np.asarray(W_proj, dtype=np.float32)
    b_proj = np.asarray(b_proj, dtype=np.float32)
    global_bias = np.asarray(global_bias, dtype=np.float32)

    xt = np.ascontiguousarray(x.reshape(TOK, C).T).astype(BF16_NP)
    wp = np.ascontiguousarray(W_proj.T).astype(BF16_NP)
    bpv = np.ascontiguousarray(b_proj[:, None])
    ebt = np.exp(global_bias).T  # [k, q]
    ebp = np.ascontiguousarray(
        ebt.reshape(NKT, 128, N).transpose(1, 0, 2)
    ).astype(BF16_NP)

    in_maps = []
    for c in range(N_CORES):
        r0 = c * 128
        wq_c = np.ascontiguousarray(W_qkv[r0:r0 + 128, :].T).astype(BF16_NP)
        wk_c = np.ascontiguousarray(W_qkv[C + r0:C + r0 + 128, :].T).astype(BF16_NP)
        vt = W_qkv[2 * C + r0:2 * C + r0 + 128, :].T  # [C, 128]
        wv_c = np.zeros((C, 130), dtype=np.float32)
        wv_c[:, 0:64] = vt[:, 0:64]
        wv_c[:, 65:129] = vt[:, 64:128]
        in_maps.append({
            "xt": xt,
            "wq": wq_c,
            "wk": wk_c,
            "wv": wv_c.astype(BF16_NP),
            "wp": wp,
            "bp": bpv,
            "eb": ebp,
        })
    return in_maps


def _assemble(results):
    full = np.empty((TOK, C), dtype=np.float32)
    for c in range(N_CORES):
        o = results[c]["out"].T  # [4*TB tokens, C], token col b*TB+i
        for b in range(B):
            full[b * N + c * TB:b * N + (c + 1) * TB, :] = (
                o[b * TB:(b + 1) * TB, :]
            )
    return full.reshape(B, N, C)


def kernel(x, W_qkv, W_proj, b_proj, global_bias):
    nc = _graph()
    in_maps = _prep_inputs(x, W_qkv, W_proj, b_proj, global_bias)
    res = run_bass_kernel_spmd(nc, in_maps, core_ids=list(range(N_CORES)))
    return _assemble(res.results)


def run_profiled(x, W_qkv, W_proj, b_proj, global_bias, **trace_kwargs):
    """Like kernel() but with NTFF profiling; returns (output, results)."""
    nc = _graph()
    in_maps = _prep_inputs(x, W_qkv, W_proj, b_proj, global_bias)
    res = run_bass_kernel_spmd(
        nc, in_maps, core_ids=list(range(N_CORES)), trace=True, **trace_kwargs
    )
    return _assemble(res.results), res
